# revision 23
# baseline (speedup 1.0000x reference)
"""AttnPooling Trainium2 Bass kernel (8-core SPMD).

Math (per graph g, head k):
  scores = tanh(h@W1+b1)@W2+b2                  [N, 8]
  e      = exp(scores)            (no max-sub; scores are O(5), safe)
  s_gk   = sum_{i in g} e_ik * h_i              [G, 8, 256]
  d_gk   = sum_{i in g} e_ik                    [G, 8]
  out_g  = (1/8) sum_k s_gk / d_gk              [G, 256]

Sharding: graphs are LPT-packed into windows of WSIZE graphs; each of
the 8 cores takes n_win windows (single SPMD program, data differs).
h is shipped ONCE, node-major bf16 (hb).  The feature-major copy that
fc1 needs is derived on-chip: PE-transpose of each 128-node tile into
PSUM (bf16), then evacuated by DVE/Pool/ACT into an fp8 DoubleRow
layout for a 2x-rate fc1 matmul.  The first K_SHIP tiles of each
window instead get their fp8 transposed copy from the host (DMA), a
knob that trades DMA bytes against PE-transpose cycles.

Per 128-node tile:
  fc1:  t1[128d, F] = W1dr.T @ ht8  (fp8 DoubleRow, one matmul)
  tanh: a1 = tanh(t1/256 + b1)               (ACT, bf16 out)
  fc2:  sco[128n, 8] = a1_slice.T @ W2       (a1 is the stationary)
  exp:  e = exp(sco + b2)                    (ACT, node-major bf16)
  E[i, g*8+k] = M[i,g] * e[i,k]              (DVE broadcast multiply)
  psumW[64, 257] += E.T @ hb[:, 0:257]       (col 256 of hb is 1.0)
Window drain:
  rc = 1/max(denom, eps); ssc = psumW[:, :256]*rc (bf16, Pool)
  outp[8, 256] = S8.T @ ssc   (S8[(g,k),g] = 1/8)  -> DMA to HBM
"""

import os
import numpy as np
import ml_dtypes

BF16 = ml_dtypes.bfloat16
FP8 = ml_dtypes.float8_e4m3
FP8_SCALE = 16.0          # h,W1 pre-scaled by 16 before fp8 cast

N_CORES = 8
WSIZE = 8           # graphs per window (8*8 heads = 64 PSUM partitions)
H = 8               # heads
GK = WSIZE * H      # 64
F_IN = 256          # in_features
D = 128             # dense dim
HB_W = 258          # 256 feat + 1 ones + 1 gidx col
MACRO = 8           # slots per macro
# tiles per window whose fp8 hT ships from host (rest: on-chip transpose)
K_SHIP = int(os.environ.get("K_SHIP", "3"))

_PROGRAM_CACHE = {}


# ----------------------------------------------------------------- host prep
def _preprocess(h, segment_ids, num_graphs):
    N = h.shape[0]
    G = int(num_graphs)
    counts = np.bincount(segment_ids, minlength=G).astype(np.int64)
    g_core = -(-G // N_CORES)
    n_win = -(-g_core // WSIZE)
    starts = np.zeros(G + 1, dtype=np.int64)
    np.cumsum(counts, out=starts[1:])

    # LPT-balance graphs into N_CORES*n_win bins of exactly WSIZE graphs
    n_bins = N_CORES * n_win
    import heapq
    heap = [(0, b, 0) for b in range(n_bins)]   # (load, bin, count)
    heapq.heapify(heap)
    bins = [[] for _ in range(n_bins)]
    for g in np.argsort(-counts, kind="stable"):
        while True:
            load, b, cnt = heapq.heappop(heap)
            if cnt < WSIZE:
                break
        bins[b].append(int(g))
        heapq.heappush(heap, (load + int(counts[g]), b, cnt + 1))
    bin_nodes = np.array([sum(counts[g] for g in bb) for bb in bins])
    t_w = int(max(1, -(-bin_nodes.max() // 128)))
    npad = n_win * t_w * 128
    B = t_w * 128
    k_ship = min(K_SHIP, t_w)

    h32 = np.ascontiguousarray(h, dtype=np.float32)
    hb_all, ht8_all = [], []
    row2graph = np.full((N_CORES, n_win * WSIZE), -1, dtype=np.int64)
    for c in range(N_CORES):
        hb = np.zeros((npad, HB_W), dtype=BF16)
        hb[:, F_IN] = 1.0
        hb[:, F_IN + 1] = 255.0          # pad rows match no window graph
        hpad = np.zeros((npad, F_IN), dtype=np.float32)
        for w in range(n_win):
            bb = bins[c * n_win + w]
            r = w * B
            for idx, g in enumerate(bb):
                row2graph[c, w * WSIZE + idx] = g
                n0, n1 = starts[g], starts[g + 1]
                nw = n1 - n0
                if nw == 0:
                    continue
                hpad[r:r + nw] = h32[n0:n1]
                hb[r:r + nw, :F_IN] = h32[n0:n1].astype(BF16)
                hb[r:r + nw, F_IN + 1] = float(idx)
                r += nw
        if k_ship > 0:
            hT = (hpad.T * FP8_SCALE).astype(FP8)
            hT = hT.reshape(F_IN, n_win, 128, t_w)
            hT = np.ascontiguousarray(
                hT.transpose(0, 1, 3, 2)).reshape(F_IN, n_win, t_w * 128)
            # ship only the first k_ship tiles of each window;
            # DoubleRow packing: ht8[p, w, i, c] = hT[i*128 + p, w, c]
            hT = np.ascontiguousarray(hT[:, :, :k_ship * 128])
            ht8 = np.ascontiguousarray(
                hT.reshape(2, D, n_win, k_ship * 128).transpose(1, 2, 0, 3))
        else:
            ht8 = np.zeros((D, n_win, 2, 1), dtype=FP8)
        hb_all.append(hb)
        ht8_all.append(ht8)
    meta = dict(G=G, g_core=g_core, n_win=n_win, t_w=t_w, npad=npad,
                k_ship=k_ship, row2graph=row2graph)
    return hb_all, ht8_all, meta


def _const_inputs(W1, b1, W2, b2):
    W1 = np.asarray(W1, dtype=np.float32)
    W2 = np.asarray(W2, dtype=np.float32)
    s8 = np.zeros((GK, WSIZE), dtype=BF16)
    for g in range(WSIZE):
        s8[g * H:(g + 1) * H, g] = 0.125
    w18 = (W1 * FP8_SCALE).astype(FP8)                # [256,128]
    # DoubleRow packing: w1dr[p, i, m] = w18[i*128 + p, m]
    w1dr = np.ascontiguousarray(
        w18.reshape(2, D, D).transpose(1, 0, 2))
    return {
        "w1dr": w1dr,                                            # [128,2,128]
        "w2": np.ascontiguousarray(W2.astype(BF16)),             # [128,8]
        "b1": np.asarray(b1, dtype=np.float32).reshape(D, 1),
        "s8": s8,                                                # [64,8]
        "iota8": np.tile(np.arange(WSIZE, dtype=BF16), (D, 1)),  # [128,8]
        "ident": np.eye(D, dtype=BF16),                          # [128,128]
    }


# ------------------------------------------------------------- device program
def _build_program(n_win, t_w, npad, k_ship, num_devices, reps=1, unroll=1):
    import concourse.bacc as bacc
    import concourse.mybir as mybir
    from concourse import tile

    dt = mybir.dt
    AF = mybir.ActivationFunctionType
    DR = mybir.MatmulPerfMode.DoubleRow
    B = t_w * 128
    KC = k_ship * 128               # shipped columns per window

    nc = bacc.Bacc("TRN2", target_bir_lowering=False, debug=False,
                   enable_asserts=False, num_devices=num_devices)

    hb_d = nc.dram_tensor("hb", [npad, HB_W], dt.bfloat16,
                          kind="ExternalInput")
    ht8_d = nc.dram_tensor("ht8", [D, n_win, 2, max(KC, 1)], dt.float8e4,
                           kind="ExternalInput")
    w1dr_d = nc.dram_tensor("w1dr", [D, 2, D], dt.float8e4,
                            kind="ExternalInput")
    w2_d = nc.dram_tensor("w2", [D, H], dt.bfloat16, kind="ExternalInput")
    b1_d = nc.dram_tensor("b1", [D, 1], dt.float32, kind="ExternalInput")
    s8_d = nc.dram_tensor("s8", [GK, WSIZE], dt.bfloat16,
                          kind="ExternalInput")
    iota8_d = nc.dram_tensor("iota8", [D, WSIZE], dt.bfloat16,
                             kind="ExternalInput")
    ident_d = nc.dram_tensor("ident", [D, D], dt.bfloat16,
                             kind="ExternalInput")
    out_d = nc.dram_tensor("out", [n_win * WSIZE, F_IN], dt.float32,
                           kind="ExternalOutput")

    # window-blocked view: [w, p, (t f)]
    hb_wv = hb_d.ap().rearrange("(w p t) f -> w p (t f)", p=128, t=t_w)

    # macro slot ranges
    macros = []
    j0 = 0
    while j0 < t_w:
        macros.append((j0, min(MACRO, t_w - j0)))
        j0 += macros[-1][1]
    # half-macro (<=4 tile) transpose-staging ranges, aligned to fc1 halves
    halves = []
    for (j0, ns) in macros:
        halves.append((j0, min(4, ns)))
        if ns > 4:
            halves.append((j0 + 4, ns - 4))

    import contextlib
    with tile.TileContext(nc) as tc:
        with (
            tc.tile_pool(name="consts", bufs=1) as cpool,
            tc.tile_pool(name="hbp", bufs=3) as hbp,
            tc.tile_pool(name="htp", bufs=2) as htp,
            tc.tile_pool(name="actp", bufs=3) as actp,
            tc.tile_pool(name="ep", bufs=3) as epool,
            tc.tile_pool(name="drainp", bufs=2) as drainp,
            tc.tile_pool(name="ps_tp", bufs=2, space="PSUM") as ps_tp,
            tc.tile_pool(name="ps_mm", bufs=2, space="PSUM") as ps_mm,
            tc.tile_pool(name="ps_sco", bufs=1, space="PSUM") as ps_sco,
            tc.tile_pool(name="ps_w", bufs=2, space="PSUM") as ps_w,
            tc.tile_pool(name="ps_out", bufs=1, space="PSUM") as ps_out,
        ):
            w1dr = cpool.tile([D, 2, D], dt.float8e4)
            w2 = cpool.tile([D, H], dt.bfloat16)
            b1 = cpool.tile([D, 1], dt.float32)
            s8 = cpool.tile([GK, WSIZE], dt.bfloat16)
            iota8 = cpool.tile([D, WSIZE], dt.bfloat16)
            ident = cpool.tile([D, D], dt.bfloat16)
            nc.sync.dma_start(out=iota8[:], in_=iota8_d.ap())
            nc.sync.dma_start(out=w1dr[:], in_=w1dr_d.ap())
            nc.sync.dma_start(out=w2[:], in_=w2_d.ap())
            nc.sync.dma_start(out=b1[:], in_=b1_d.ap())
            nc.sync.dma_start(out=s8[:], in_=s8_d.ap())
            nc.sync.dma_start(out=ident[:], in_=ident_d.ap())

            loop_cm = (tc.For_i(0, reps // unroll, 1)
                       if reps // unroll > 1 else contextlib.nullcontext())
            with loop_cm:
              for w in [wi for _ in range(unroll) for wi in range(n_win)]:
                hb_sb = hbp.tile([128, t_w, HB_W], dt.bfloat16, tag="hb")
                th = t_w // 2
                nc.gpsimd.dma_start(out=hb_sb[:, :th, :],
                                    in_=hb_wv[w][:, :th * HB_W])
                nc.gpsimd.dma_start(out=hb_sb[:, th:, :],
                                    in_=hb_wv[w][:, th * HB_W:])
                ht8_sb = htp.tile([D, 2, B], dt.float8e4, tag="h8")
                if k_ship > 0:
                    nc.sync.dma_start(
                        out=ht8_sb[:, :, :KC],
                        in_=ht8_d.ap()[:, w, :, :])

                def emit_tp(hidx):
                    """PE-transpose + evacuate half-macro hidx (j >= k_ship
                    tiles only)."""
                    if hidx >= len(halves):
                        return
                    f0, nsh = halves[hidx]
                    jlo = max(f0, k_ship)
                    if jlo >= f0 + nsh:
                        return
                    nt = f0 + nsh - jlo          # tiles to transpose
                    psT = ps_tp.tile([128, 2, 512], dt.bfloat16, tag="tp")
                    for jj in range(nt):
                        j = jlo + jj
                        nc.tensor.transpose(
                            psT[:, 0, jj * 128:(jj + 1) * 128],
                            hb_sb[:, j, 0:D], ident[:])
                        nc.tensor.transpose(
                            psT[:, 1, jj * 128:(jj + 1) * 128],
                            hb_sb[:, j, D:2 * D], ident[:])
                    # evacuate PSUM bf16 -> SBUF fp8 (x16); DVE + ACT only
                    # (GPSIMD/Pool cannot access PSUM)
                    cw = nt * 128
                    s1 = ((cw * 8) // 16) & ~31          # DVE share
                    dst0 = jlo * 128
                    if s1 > 0:
                        nc.vector.tensor_scalar_mul(
                            ht8_sb[:, :, dst0:dst0 + s1],
                            psT[:, :, :s1], FP8_SCALE)
                    if cw > s1:
                        nc.scalar.activation(
                            ht8_sb[:, :, dst0 + s1:dst0 + cw],
                            psT[:, :, s1:cw], AF.Copy, scale=FP8_SCALE)

                emit_tp(0)
                hidx = 0
                psw = ps_w.tile([GK, F_IN + 1], dt.float32)
                for (j0, ns) in macros:
                    # score path in halves of <=512 nodes
                    sco = ps_sco.tile([128, MACRO * H], dt.float32,
                                      tag="sco")
                    n_half = (ns * 128 + 511) // 512
                    for hh in range(n_half):
                        emit_tp(hidx + 1)
                        hidx += 1
                        f0 = j0 * 128 + hh * 512
                        fw = min(512, (j0 + ns) * 128 - f0)
                        t1 = ps_mm.tile([D, 512], dt.float32, tag="t1")
                        nc.tensor.matmul(t1[:, :fw], w1dr[:],
                                         ht8_sb[:, :, f0:f0 + fw],
                                         start=True, stop=True,
                                         perf_mode=DR)
                        a1 = actp.tile([D, 512], dt.bfloat16, tag="a1")
                        nc.scalar.activation(a1[:, :fw], t1[:, :fw],
                                             AF.Tanh, bias=b1[:],
                                             scale=1.0 / (FP8_SCALE ** 2))
                        for jj in range(fw // 128):
                            j = hh * 4 + jj     # slot within macro
                            nc.tensor.matmul(
                                sco[:, j * H:(j + 1) * H],
                                a1[:, jj * 128:(jj + 1) * 128], w2[:],
                                start=True, stop=True)
                    e_sb = epool.tile([128, MACRO * H], dt.bfloat16,
                                      tag="e")
                    nc.scalar.activation(e_sb[:, :ns * H], sco[:, :ns * H],
                                         AF.Exp)
                    msk = epool.tile([128, MACRO * WSIZE], dt.bfloat16,
                                     tag="M")
                    g_b = hb_sb[:, j0:j0 + ns,
                                F_IN + 1:F_IN + 2].broadcast_to(
                                    (128, ns, WSIZE))
                    i_b = iota8[:].unsqueeze(1).broadcast_to(
                        (128, ns, WSIZE))
                    nc.vector.tensor_tensor(
                        msk[:, :ns * WSIZE].rearrange(
                            "p (j g) -> p j g", g=WSIZE),
                        g_b, i_b, mybir.AluOpType.is_equal)
                    em = epool.tile([128, MACRO * GK], dt.bfloat16,
                                    tag="E")
                    e_b = e_sb[:, :ns * H].rearrange(
                        "p (j k) -> p j k", k=H).unsqueeze(2).broadcast_to(
                            (128, ns, WSIZE, H))
                    m_b = msk[:, :ns * WSIZE].rearrange(
                        "p (j g) -> p j g", g=WSIZE).unsqueeze(3).broadcast_to(
                            (128, ns, WSIZE, H))
                    nc.vector.tensor_mul(
                        em[:, :ns * GK].rearrange(
                            "p (j g k) -> p j g k", g=WSIZE, k=H),
                        m_b, e_b)
                    for jj in range(ns):
                        j = j0 + jj
                        nc.tensor.matmul(
                            psw[:],
                            em[:, jj * GK:(jj + 1) * GK],
                            hb_sb[:, j, :F_IN + 1],
                            start=(j == 0), stop=(j == t_w - 1))

                # drain window
                dcl = drainp.tile([GK, 1], dt.float32, tag="dcl")
                nc.vector.tensor_scalar_max(dcl[:], psw[:, F_IN:F_IN + 1],
                                            1e-30)
                rc = drainp.tile([GK, 1], dt.float32, tag="rc")
                nc.vector.reciprocal(rc[:], dcl[:])
                ssc = drainp.tile([GK, F_IN], dt.bfloat16, tag="ssc")
                nc.vector.tensor_scalar_mul(ssc[:], psw[:, :F_IN], rc[:])
                outp = ps_out.tile([WSIZE, F_IN], dt.float32)
                nc.tensor.matmul(outp[:], s8[:], ssc[:], start=True,
                                 stop=True)
                out_sb = drainp.tile([WSIZE, F_IN], dt.float32, tag="osb")
                nc.vector.tensor_copy(out_sb[:], outp[:])
                nc.scalar.dma_start(
                    out=out_d.ap()[w * WSIZE:(w + 1) * WSIZE, :],
                    in_=out_sb[:])

    nc.compile()
    return nc


# ---------------------------------------------------------------- jit runner
class _Runner:
    """Persistent sharded jit wrapper around the compiled Bass program.

    Mirrors bass2jax.run_bass_via_pjrt's multi-core path, but keeps the
    jitted callable and device-resident inputs so repeated executions (for
    timing) skip retrace/recompile/re-transfer.
    """

    def __init__(self, nc):
        import jax
        import concourse.mybir as mybir
        from concourse import bass2jax
        from jax.experimental.shard_map import shard_map
        from jax.sharding import Mesh, PartitionSpec

        bass2jax.install_neuronx_cc_hook()
        self.jax = jax
        part_name = (nc.partition_id_tensor.name
                     if nc.partition_id_tensor else None)
        in_names, out_names, out_avals, zero_outs = [], [], [], []
        for alloc in nc.m.functions[0].allocations:
            if not isinstance(alloc, mybir.MemoryLocationSet):
                continue
            name = alloc.memorylocations[0].name
            if alloc.kind == "ExternalInput":
                if name == part_name:
                    continue
                in_names.append(name)
            elif alloc.kind == "ExternalOutput":
                out_names.append(name)
                shape = tuple(alloc.tensor_shape)
                dtype = mybir.dt.np(alloc.dtype)
                out_avals.append(jax.core.ShapedArray(shape, dtype))
                zero_outs.append(np.zeros(shape, dtype))
        n_params = len(in_names)
        self.in_names = list(in_names)
        self.out_names = out_names
        self.out_avals = out_avals
        self.zero_outs = zero_outs

        bind_names = list(in_names) + list(out_names)
        if part_name is not None:
            bind_names.append(part_name)

        def _body(*args):
            operands = list(args)
            if part_name is not None:
                operands.append(bass2jax.partition_id_tensor())
            outs = bass2jax._bass_exec_p.bind(
                *operands,
                out_avals=tuple(out_avals),
                in_names=tuple(bind_names),
                out_names=tuple(out_names),
                lowering_input_output_aliases=(),
                sim_require_finite=True,
                sim_require_nnan=True,
                nc=nc,
            )
            return tuple(outs)

        devices = jax.devices()[:N_CORES]
        self.mesh = Mesh(np.asarray(devices), ("core",))
        self.pspec = PartitionSpec("core")
        in_specs = (self.pspec,) * (n_params + len(out_names))
        out_specs = (self.pspec,) * len(out_names)
        donate = tuple(range(n_params, n_params + len(out_names)))
        self.sharded = jax.jit(
            shard_map(_body, mesh=self.mesh, in_specs=in_specs,
                      out_specs=out_specs, check_rep=False),
            donate_argnums=donate, keep_unused=True)

    def put_inputs(self, in_maps):
        import jax
        from jax.sharding import NamedSharding
        sh = NamedSharding(self.mesh, self.pspec)
        self.dev_in = [
            jax.device_put(
                np.concatenate([np.asarray(m[name]) for m in in_maps],
                               axis=0), sh)
            for name in self.in_names]

    def run(self, block=True):
        import jax
        from jax.sharding import NamedSharding
        sh = NamedSharding(self.mesh, self.pspec)
        zeros = [jax.device_put(
            np.zeros((N_CORES * z.shape[0], *z.shape[1:]), z.dtype), sh)
            for z in self.zero_outs]
        out = self.sharded(*self.dev_in, *zeros)
        if block:
            jax.block_until_ready(out)
        return out

    def timed_burst(self, n):
        """Dispatch n executions async, block at the end; return wall s."""
        import jax
        import time as _t
        t0 = _t.perf_counter()
        out = None
        for _ in range(n):
            out = self.run(block=False)
        jax.block_until_ready(out)
        return _t.perf_counter() - t0

    def results(self, out_arrs):
        return [
            {name: np.asarray(out_arrs[i]).reshape(
                N_CORES, *self.out_avals[i].shape)[c]
             for i, name in enumerate(self.out_names)}
            for c in range(N_CORES)]


_RUNNER_CACHE = {}


# ------------------------------------------------------------------- kernel()
def kernel(h, segment_ids, W1, b1, W2, b2, num_graphs):
    h = np.asarray(h)
    segment_ids = np.asarray(segment_ids)
    G = int(num_graphs)

    hb_all, ht8_all, meta = _preprocess(h, segment_ids, G)
    consts = _const_inputs(W1, b1, W2, b2)

    key = (meta["n_win"], meta["t_w"], meta["npad"], meta["k_ship"])
    if key not in _RUNNER_CACHE:
        nc = _build_program(meta["n_win"], meta["t_w"], meta["npad"],
                            meta["k_ship"], N_CORES)
        _RUNNER_CACHE[key] = _Runner(nc)
    runner = _RUNNER_CACHE[key]

    in_maps = []
    for c in range(N_CORES):
        m = {"hb": hb_all[c], "ht8": ht8_all[c]}
        m.update(consts)
        in_maps.append(m)
    runner.put_inputs(in_maps)

    out_arrs = runner.run()   # first call compiles NEFF
    reps = int(os.environ.get("KERNEL_TIME_REPS", "0"))
    if reps:
        n_lo, n_hi = 2, 2 + reps
        t_lo = min(runner.timed_burst(n_lo) for _ in range(3))
        t_hi = min(runner.timed_burst(n_hi) for _ in range(3))
        slope = (t_hi - t_lo) / (n_hi - n_lo)
        print(f"burst timing: n={n_lo}: {t_lo*1e3:.2f} ms, "
              f"n={n_hi}: {t_hi*1e3:.2f} ms")
        print(f"HW exec time: {int(slope * 1e9)} ns")

    res = runner.results(out_arrs)
    out = np.zeros((G, F_IN), dtype=np.float32)
    r2g = meta["row2graph"]
    for c in range(N_CORES):
        valid = r2g[c] >= 0
        out[r2g[c][valid]] = res[c]["out"][valid]
    return out


# revision 25
# speedup vs baseline: 1.0863x; 1.0863x over previous
"""AttnPooling Trainium2 Bass kernel (8-core SPMD).

Math (per graph g, head k):
  scores = tanh(h@W1+b1)@W2+b2                  [N, 8]
  e      = exp(scores)            (no max-sub; scores are O(5), safe)
  s_gk   = sum_{i in g} e_ik * h_i              [G, 8, 256]
  d_gk   = sum_{i in g} e_ik                    [G, 8]
  out_g  = (1/8) sum_k s_gk / d_gk              [G, 256]

Sharding: graphs are LPT-packed into windows of WSIZE graphs; each of
the 8 cores takes n_win windows (single SPMD program, data differs).
h is shipped ONCE, node-major bf16 (hb).  The feature-major copy that
fc1 needs is derived on-chip: PE-transpose of each 128-node tile into
PSUM (bf16), then evacuated by DVE/Pool/ACT into an fp8 DoubleRow
layout for a 2x-rate fc1 matmul.  The first K_SHIP tiles of each
window instead get their fp8 transposed copy from the host (DMA), a
knob that trades DMA bytes against PE-transpose cycles.

Per 128-node tile:
  fc1:  t1[128d, F] = W1dr.T @ ht8  (fp8 DoubleRow, one matmul)
  tanh: a1 = tanh(t1/256 + b1)               (ACT, bf16 out)
  fc2:  sco[128n, 8] = a1_slice.T @ W2       (a1 is the stationary)
  exp:  e = exp(sco + b2)                    (ACT, node-major bf16)
  E[i, g*8+k] = M[i,g] * e[i,k]              (DVE broadcast multiply)
  psumW[64, 257] += E.T @ hb[:, 0:257]       (col 256 of hb is 1.0)
Window drain:
  rc = 1/max(denom, eps); ssc = psumW[:, :256]*rc (bf16, Pool)
  outp[8, 256] = S8.T @ ssc   (S8[(g,k),g] = 1/8)  -> DMA to HBM
"""

import os
import numpy as np
import ml_dtypes

BF16 = ml_dtypes.bfloat16
FP8 = ml_dtypes.float8_e4m3
FP8_SCALE = 16.0          # h,W1 pre-scaled by 16 before fp8 cast

N_CORES = 8
WSIZE = 8           # graphs per window (8*8 heads = 64 PSUM partitions)
H = 8               # heads
GK = WSIZE * H      # 64
F_IN = 256          # in_features
D = 128             # dense dim
HB_W = 258          # 256 feat + 1 ones + 1 gidx col
MACRO = 8           # slots per macro
# tiles per window whose fp8 hT ships from host (rest: on-chip transpose)
K_SHIP = int(os.environ.get("K_SHIP", "3"))

_PROGRAM_CACHE = {}


# ----------------------------------------------------------------- host prep
def _preprocess(h, segment_ids, num_graphs):
    N = h.shape[0]
    G = int(num_graphs)
    counts = np.bincount(segment_ids, minlength=G).astype(np.int64)
    g_core = -(-G // N_CORES)
    n_win = -(-g_core // WSIZE)
    starts = np.zeros(G + 1, dtype=np.int64)
    np.cumsum(counts, out=starts[1:])

    # LPT-balance graphs into N_CORES*n_win bins of exactly WSIZE graphs
    n_bins = N_CORES * n_win
    import heapq
    heap = [(0, b, 0) for b in range(n_bins)]   # (load, bin, count)
    heapq.heapify(heap)
    bins = [[] for _ in range(n_bins)]
    for g in np.argsort(-counts, kind="stable"):
        while True:
            load, b, cnt = heapq.heappop(heap)
            if cnt < WSIZE:
                break
        bins[b].append(int(g))
        heapq.heappush(heap, (load + int(counts[g]), b, cnt + 1))
    bin_nodes = np.array([sum(counts[g] for g in bb) for bb in bins])
    t_w = int(max(1, -(-bin_nodes.max() // 128)))
    npad = n_win * t_w * 128
    B = t_w * 128
    k_ship = min(K_SHIP, t_w)

    h32 = np.ascontiguousarray(h, dtype=np.float32)
    hb_all, ht8_all = [], []
    row2graph = np.full((N_CORES, n_win * WSIZE), -1, dtype=np.int64)
    for c in range(N_CORES):
        hb = np.zeros((npad, HB_W), dtype=BF16)
        hb[:, F_IN] = 1.0
        hb[:, F_IN + 1] = 255.0          # pad rows match no window graph
        hpad = np.zeros((npad, F_IN), dtype=np.float32)
        for w in range(n_win):
            bb = bins[c * n_win + w]
            r = w * B
            for idx, g in enumerate(bb):
                row2graph[c, w * WSIZE + idx] = g
                n0, n1 = starts[g], starts[g + 1]
                nw = n1 - n0
                if nw == 0:
                    continue
                hpad[r:r + nw] = h32[n0:n1]
                hb[r:r + nw, :F_IN] = h32[n0:n1].astype(BF16)
                hb[r:r + nw, F_IN + 1] = float(idx)
                r += nw
        if k_ship > 0:
            hT = (hpad.T * FP8_SCALE).astype(FP8)
            hT = hT.reshape(F_IN, n_win, 128, t_w)
            hT = np.ascontiguousarray(
                hT.transpose(0, 1, 3, 2)).reshape(F_IN, n_win, t_w * 128)
            # ship only the first k_ship tiles of each window;
            # DoubleRow packing: ht8[p, w, i, c] = hT[i*128 + p, w, c]
            hT = np.ascontiguousarray(hT[:, :, :k_ship * 128])
            ht8 = np.ascontiguousarray(
                hT.reshape(2, D, n_win, k_ship * 128).transpose(1, 2, 0, 3))
        else:
            ht8 = np.zeros((D, n_win, 2, 1), dtype=FP8)
        hb_all.append(hb)
        ht8_all.append(ht8)
    meta = dict(G=G, g_core=g_core, n_win=n_win, t_w=t_w, npad=npad,
                k_ship=k_ship, row2graph=row2graph)
    return hb_all, ht8_all, meta


def _const_inputs(W1, b1, W2, b2):
    W1 = np.asarray(W1, dtype=np.float32)
    W2 = np.asarray(W2, dtype=np.float32)
    s8 = np.zeros((GK, WSIZE), dtype=BF16)
    for g in range(WSIZE):
        s8[g * H:(g + 1) * H, g] = 0.125
    w18 = (W1 * FP8_SCALE).astype(FP8)                # [256,128]
    # DoubleRow packing: w1dr[p, i, m] = w18[i*128 + p, m]
    w1dr = np.ascontiguousarray(
        w18.reshape(2, D, D).transpose(1, 0, 2))
    return {
        "w1dr": w1dr,                                            # [128,2,128]
        "w2": np.ascontiguousarray(W2.astype(BF16)),             # [128,8]
        "b1": np.asarray(b1, dtype=np.float32).reshape(D, 1),
        "s8": s8,                                                # [64,8]
        "iota8": np.tile(np.arange(WSIZE, dtype=BF16), (D, 1)),  # [128,8]
        "ident": np.eye(D, dtype=BF16),                          # [128,128]
    }


# ------------------------------------------------------------- device program
def _build_program(n_win, t_w, npad, k_ship, num_devices, reps=1, unroll=1):
    import concourse.bacc as bacc
    import concourse.mybir as mybir
    from concourse import tile

    dt = mybir.dt
    AF = mybir.ActivationFunctionType
    DR = mybir.MatmulPerfMode.DoubleRow
    B = t_w * 128
    KC = k_ship * 128               # shipped columns per window

    nc = bacc.Bacc("TRN2", target_bir_lowering=False, debug=False,
                   enable_asserts=False, num_devices=num_devices)

    hb_d = nc.dram_tensor("hb", [npad, HB_W], dt.bfloat16,
                          kind="ExternalInput")
    ht8_d = nc.dram_tensor("ht8", [D, n_win, 2, max(KC, 1)], dt.float8e4,
                           kind="ExternalInput")
    w1dr_d = nc.dram_tensor("w1dr", [D, 2, D], dt.float8e4,
                            kind="ExternalInput")
    w2_d = nc.dram_tensor("w2", [D, H], dt.bfloat16, kind="ExternalInput")
    b1_d = nc.dram_tensor("b1", [D, 1], dt.float32, kind="ExternalInput")
    s8_d = nc.dram_tensor("s8", [GK, WSIZE], dt.bfloat16,
                          kind="ExternalInput")
    iota8_d = nc.dram_tensor("iota8", [D, WSIZE], dt.bfloat16,
                             kind="ExternalInput")
    ident_d = nc.dram_tensor("ident", [D, D], dt.bfloat16,
                             kind="ExternalInput")
    out_d = nc.dram_tensor("out", [n_win * WSIZE, F_IN], dt.float32,
                           kind="ExternalOutput")

    # window-blocked view: [w, p, (t f)]
    hb_wv = hb_d.ap().rearrange("(w p t) f -> w p (t f)", p=128, t=t_w)

    # macro slot ranges
    macros = []
    j0 = 0
    while j0 < t_w:
        macros.append((j0, min(MACRO, t_w - j0)))
        j0 += macros[-1][1]
    # half-macro (<=4 tile) transpose-staging ranges, aligned to fc1 halves
    halves = []
    for (j0, ns) in macros:
        halves.append((j0, min(4, ns)))
        if ns > 4:
            halves.append((j0 + 4, ns - 4))

    import contextlib
    with tile.TileContext(nc) as tc:
        with (
            tc.tile_pool(name="consts", bufs=1) as cpool,
            tc.tile_pool(name="hbp", bufs=3) as hbp,
            tc.tile_pool(name="htp", bufs=2) as htp,
            tc.tile_pool(name="actp", bufs=3) as actp,
            tc.tile_pool(name="ep", bufs=3) as epool,
            tc.tile_pool(name="drainp", bufs=2) as drainp,
            tc.tile_pool(name="ps_tp", bufs=2, space="PSUM") as ps_tp,
            tc.tile_pool(name="ps_mm", bufs=2, space="PSUM") as ps_mm,
            tc.tile_pool(name="ps_sco", bufs=1, space="PSUM") as ps_sco,
            tc.tile_pool(name="ps_w", bufs=2, space="PSUM") as ps_w,
            tc.tile_pool(name="ps_out", bufs=1, space="PSUM") as ps_out,
        ):
            w1dr = cpool.tile([D, 2, D], dt.float8e4)
            w2 = cpool.tile([D, H], dt.bfloat16)
            b1 = cpool.tile([D, 1], dt.float32)
            s8 = cpool.tile([GK, WSIZE], dt.bfloat16)
            iota8 = cpool.tile([D, WSIZE], dt.bfloat16)
            ident = cpool.tile([D, D], dt.bfloat16)
            nc.sync.dma_start(out=iota8[:], in_=iota8_d.ap())
            nc.sync.dma_start(out=w1dr[:], in_=w1dr_d.ap())
            nc.sync.dma_start(out=w2[:], in_=w2_d.ap())
            nc.sync.dma_start(out=b1[:], in_=b1_d.ap())
            nc.sync.dma_start(out=s8[:], in_=s8_d.ap())
            nc.sync.dma_start(out=ident[:], in_=ident_d.ap())

            loop_cm = (tc.For_i(0, reps // unroll, 1)
                       if reps // unroll > 1 else contextlib.nullcontext())
            with loop_cm:
              for w in [wi for _ in range(unroll) for wi in range(n_win)]:
                hb_sb = hbp.tile([128, t_w, HB_W], dt.bfloat16, tag="hb")
                th = t_w // 2
                nc.gpsimd.dma_start(out=hb_sb[:, :th, :],
                                    in_=hb_wv[w][:, :th * HB_W])
                nc.gpsimd.dma_start(out=hb_sb[:, th:, :],
                                    in_=hb_wv[w][:, th * HB_W:])
                ht8_hs = {}                    # per-half fp8 hT tiles

                def emit_tp(hidx):
                    """Stage half-macro hidx's fp8 hT tile: DMA for shipped
                    tiles, PE-transpose + evacuate for the rest."""
                    if hidx >= len(halves):
                        return
                    f0, nsh = halves[hidx]
                    ht8_h = htp.tile([D, 2, 512], dt.float8e4, tag="h8")
                    ht8_hs[hidx] = ht8_h
                    jlo = max(f0, min(k_ship, f0 + nsh))
                    if jlo > f0:                 # shipped columns
                        sw = (jlo - f0) * 128
                        nc.sync.dma_start(
                            out=ht8_h[:, :, :sw],
                            in_=ht8_d.ap()[:, w, :,
                                           f0 * 128:f0 * 128 + sw])
                    if jlo >= f0 + nsh:
                        return
                    nt = f0 + nsh - jlo          # tiles to transpose
                    psT = ps_tp.tile([128, 2, 512], dt.bfloat16, tag="tp")
                    for jj in range(nt):
                        j = jlo + jj
                        nc.tensor.transpose(
                            psT[:, 0, jj * 128:(jj + 1) * 128],
                            hb_sb[:, j, 0:D], ident[:])
                        nc.tensor.transpose(
                            psT[:, 1, jj * 128:(jj + 1) * 128],
                            hb_sb[:, j, D:2 * D], ident[:])
                    # evacuate PSUM bf16 -> SBUF fp8 (x16); DVE + ACT only
                    # (GPSIMD/Pool cannot access PSUM)
                    cw = nt * 128
                    s1 = ((cw * 8) // 16) & ~31          # DVE share
                    dst0 = (jlo - f0) * 128
                    if s1 > 0:
                        nc.vector.tensor_scalar_mul(
                            ht8_h[:, :, dst0:dst0 + s1],
                            psT[:, :, :s1], FP8_SCALE)
                    if cw > s1:
                        nc.scalar.activation(
                            ht8_h[:, :, dst0 + s1:dst0 + cw],
                            psT[:, :, s1:cw], AF.Copy, scale=FP8_SCALE)

                emit_tp(0)
                hidx = 0
                psw = ps_w.tile([GK, F_IN + 1], dt.float32)
                for (j0, ns) in macros:
                    # score path in halves of <=512 nodes
                    sco = ps_sco.tile([128, MACRO * H], dt.float32,
                                      tag="sco")
                    n_half = (ns * 128 + 511) // 512
                    for hh in range(n_half):
                        emit_tp(hidx + 1)
                        cur = hidx
                        hidx += 1
                        f0 = j0 * 128 + hh * 512
                        fw = min(512, (j0 + ns) * 128 - f0)
                        t1 = ps_mm.tile([D, 512], dt.float32, tag="t1")
                        nc.tensor.matmul(t1[:, :fw], w1dr[:],
                                         ht8_hs[cur][:, :, :fw],
                                         start=True, stop=True,
                                         perf_mode=DR)
                        a1 = actp.tile([D, 512], dt.bfloat16, tag="a1")
                        nc.scalar.activation(a1[:, :fw], t1[:, :fw],
                                             AF.Tanh, bias=b1[:],
                                             scale=1.0 / (FP8_SCALE ** 2))
                        for jj in range(fw // 128):
                            j = hh * 4 + jj     # slot within macro
                            nc.tensor.matmul(
                                sco[:, j * H:(j + 1) * H],
                                a1[:, jj * 128:(jj + 1) * 128], w2[:],
                                start=True, stop=True)
                    e_sb = epool.tile([128, MACRO * H], dt.bfloat16,
                                      tag="e")
                    nc.scalar.activation(e_sb[:, :ns * H], sco[:, :ns * H],
                                         AF.Exp)
                    msk = epool.tile([128, MACRO * WSIZE], dt.bfloat16,
                                     tag="M")
                    g_b = hb_sb[:, j0:j0 + ns,
                                F_IN + 1:F_IN + 2].broadcast_to(
                                    (128, ns, WSIZE))
                    i_b = iota8[:].unsqueeze(1).broadcast_to(
                        (128, ns, WSIZE))
                    nc.vector.tensor_tensor(
                        msk[:, :ns * WSIZE].rearrange(
                            "p (j g) -> p j g", g=WSIZE),
                        g_b, i_b, mybir.AluOpType.is_equal)
                    em = epool.tile([128, MACRO * GK], dt.bfloat16,
                                    tag="E")
                    e_b = e_sb[:, :ns * H].rearrange(
                        "p (j k) -> p j k", k=H).unsqueeze(2).broadcast_to(
                            (128, ns, WSIZE, H))
                    m_b = msk[:, :ns * WSIZE].rearrange(
                        "p (j g) -> p j g", g=WSIZE).unsqueeze(3).broadcast_to(
                            (128, ns, WSIZE, H))
                    nc.vector.tensor_mul(
                        em[:, :ns * GK].rearrange(
                            "p (j g k) -> p j g k", g=WSIZE, k=H),
                        m_b, e_b)
                    for jj in range(ns):
                        j = j0 + jj
                        nc.tensor.matmul(
                            psw[:],
                            em[:, jj * GK:(jj + 1) * GK],
                            hb_sb[:, j, :F_IN + 1],
                            start=(j == 0), stop=(j == t_w - 1))

                # drain window
                dcl = drainp.tile([GK, 1], dt.float32, tag="dcl")
                nc.vector.tensor_scalar_max(dcl[:], psw[:, F_IN:F_IN + 1],
                                            1e-30)
                rc = drainp.tile([GK, 1], dt.float32, tag="rc")
                nc.vector.reciprocal(rc[:], dcl[:])
                ssc = drainp.tile([GK, F_IN], dt.bfloat16, tag="ssc")
                nc.vector.tensor_scalar_mul(ssc[:], psw[:, :F_IN], rc[:])
                outp = ps_out.tile([WSIZE, F_IN], dt.float32)
                nc.tensor.matmul(outp[:], s8[:], ssc[:], start=True,
                                 stop=True)
                out_sb = drainp.tile([WSIZE, F_IN], dt.float32, tag="osb")
                nc.vector.tensor_copy(out_sb[:], outp[:])
                nc.scalar.dma_start(
                    out=out_d.ap()[w * WSIZE:(w + 1) * WSIZE, :],
                    in_=out_sb[:])

    nc.compile()
    return nc


# ---------------------------------------------------------------- jit runner
class _Runner:
    """Persistent sharded jit wrapper around the compiled Bass program.

    Mirrors bass2jax.run_bass_via_pjrt's multi-core path, but keeps the
    jitted callable and device-resident inputs so repeated executions (for
    timing) skip retrace/recompile/re-transfer.
    """

    def __init__(self, nc):
        import jax
        import concourse.mybir as mybir
        from concourse import bass2jax
        from jax.experimental.shard_map import shard_map
        from jax.sharding import Mesh, PartitionSpec

        bass2jax.install_neuronx_cc_hook()
        self.jax = jax
        part_name = (nc.partition_id_tensor.name
                     if nc.partition_id_tensor else None)
        in_names, out_names, out_avals, zero_outs = [], [], [], []
        for alloc in nc.m.functions[0].allocations:
            if not isinstance(alloc, mybir.MemoryLocationSet):
                continue
            name = alloc.memorylocations[0].name
            if alloc.kind == "ExternalInput":
                if name == part_name:
                    continue
                in_names.append(name)
            elif alloc.kind == "ExternalOutput":
                out_names.append(name)
                shape = tuple(alloc.tensor_shape)
                dtype = mybir.dt.np(alloc.dtype)
                out_avals.append(jax.core.ShapedArray(shape, dtype))
                zero_outs.append(np.zeros(shape, dtype))
        n_params = len(in_names)
        self.in_names = list(in_names)
        self.out_names = out_names
        self.out_avals = out_avals
        self.zero_outs = zero_outs

        bind_names = list(in_names) + list(out_names)
        if part_name is not None:
            bind_names.append(part_name)

        def _body(*args):
            operands = list(args)
            if part_name is not None:
                operands.append(bass2jax.partition_id_tensor())
            outs = bass2jax._bass_exec_p.bind(
                *operands,
                out_avals=tuple(out_avals),
                in_names=tuple(bind_names),
                out_names=tuple(out_names),
                lowering_input_output_aliases=(),
                sim_require_finite=True,
                sim_require_nnan=True,
                nc=nc,
            )
            return tuple(outs)

        devices = jax.devices()[:N_CORES]
        self.mesh = Mesh(np.asarray(devices), ("core",))
        self.pspec = PartitionSpec("core")
        in_specs = (self.pspec,) * (n_params + len(out_names))
        out_specs = (self.pspec,) * len(out_names)
        donate = tuple(range(n_params, n_params + len(out_names)))
        self.sharded = jax.jit(
            shard_map(_body, mesh=self.mesh, in_specs=in_specs,
                      out_specs=out_specs, check_rep=False),
            donate_argnums=donate, keep_unused=True)

    def put_inputs(self, in_maps):
        import jax
        from jax.sharding import NamedSharding
        sh = NamedSharding(self.mesh, self.pspec)
        self.dev_in = [
            jax.device_put(
                np.concatenate([np.asarray(m[name]) for m in in_maps],
                               axis=0), sh)
            for name in self.in_names]

    def run(self, block=True):
        import jax
        from jax.sharding import NamedSharding
        sh = NamedSharding(self.mesh, self.pspec)
        zeros = [jax.device_put(
            np.zeros((N_CORES * z.shape[0], *z.shape[1:]), z.dtype), sh)
            for z in self.zero_outs]
        out = self.sharded(*self.dev_in, *zeros)
        if block:
            jax.block_until_ready(out)
        return out

    def timed_burst(self, n):
        """Dispatch n executions async, block at the end; return wall s."""
        import jax
        import time as _t
        t0 = _t.perf_counter()
        out = None
        for _ in range(n):
            out = self.run(block=False)
        jax.block_until_ready(out)
        return _t.perf_counter() - t0

    def results(self, out_arrs):
        return [
            {name: np.asarray(out_arrs[i]).reshape(
                N_CORES, *self.out_avals[i].shape)[c]
             for i, name in enumerate(self.out_names)}
            for c in range(N_CORES)]


_RUNNER_CACHE = {}


# ------------------------------------------------------------------- kernel()
def kernel(h, segment_ids, W1, b1, W2, b2, num_graphs):
    h = np.asarray(h)
    segment_ids = np.asarray(segment_ids)
    G = int(num_graphs)

    hb_all, ht8_all, meta = _preprocess(h, segment_ids, G)
    consts = _const_inputs(W1, b1, W2, b2)

    key = (meta["n_win"], meta["t_w"], meta["npad"], meta["k_ship"])
    if key not in _RUNNER_CACHE:
        nc = _build_program(meta["n_win"], meta["t_w"], meta["npad"],
                            meta["k_ship"], N_CORES)
        _RUNNER_CACHE[key] = _Runner(nc)
    runner = _RUNNER_CACHE[key]

    in_maps = []
    for c in range(N_CORES):
        m = {"hb": hb_all[c], "ht8": ht8_all[c]}
        m.update(consts)
        in_maps.append(m)
    runner.put_inputs(in_maps)

    out_arrs = runner.run()   # first call compiles NEFF
    reps = int(os.environ.get("KERNEL_TIME_REPS", "0"))
    if reps:
        n_lo, n_hi = 2, 2 + reps
        t_lo = min(runner.timed_burst(n_lo) for _ in range(3))
        t_hi = min(runner.timed_burst(n_hi) for _ in range(3))
        slope = (t_hi - t_lo) / (n_hi - n_lo)
        print(f"burst timing: n={n_lo}: {t_lo*1e3:.2f} ms, "
              f"n={n_hi}: {t_hi*1e3:.2f} ms")
        print(f"HW exec time: {int(slope * 1e9)} ns")

    res = runner.results(out_arrs)
    out = np.zeros((G, F_IN), dtype=np.float32)
    r2g = meta["row2graph"]
    for c in range(N_CORES):
        valid = r2g[c] >= 0
        out[r2g[c][valid]] = res[c]["out"][valid]
    return out


# revision 49
# speedup vs baseline: 1.6443x; 1.5137x over previous
"""AttnPooling Trainium2 Bass kernel (8-core SPMD).

Math (per graph g, head k):
  scores = tanh(h@W1+b1)@W2+b2                  [N, 8]
  e      = exp(scores)            (no max-sub; scores are O(5), safe)
  s_gk   = sum_{i in g} e_ik * h_i              [G, 8, 256]
  d_gk   = sum_{i in g} e_ik                    [G, 8]
  out_g  = (1/8) sum_k s_gk / d_gk              [G, 256]

Sharding: graphs are LPT-packed into windows of 16 (16 graphs x 8 heads
= 128 PSUM partitions); each of the 8 cores takes n_win windows; window
node counts are padded to a multiple of 128 so every tile belongs to
exactly one window and the program structure is identical across cores
(only the data differs -> single SPMD program).

h ships in two layouts: node-major bf16 (hb, for the weighted-sum
matmul; bf16 needed for the 2e-2 output tolerance) and feature-major
fp8e4m3 x16-scaled DoubleRow-packed (ht8, score path only, where fp8
noise only perturbs attention weights by ~1%).  This is DMA-bound at
~39.5 MB/core; fp8 halves the score-path bytes vs the bf16 baseline.

Per 128-node tile:
  fc1:  t1[128d, F] = W1dr.T @ ht8   (ONE fp8 DoubleRow matmul,
        256-deep contraction at 0.5 cyc/row)
  tanh: a1 = tanh(t1/256 + b1)               (ACT, bf16 out)
  fc2:  sco[128n, 8] = a1_slice.T @ W2       (a1 is the stationary)
  exp:  e = exp(sco + b2)                    (ACT, node-major bf16)
  E[i, g*8+k] = M[i,g] * e[i,k]              (DVE broadcast multiply)
  psumW[128, 257] += E.T @ hb[:, 0:257]      (col 256 of hb is 1.0 -> denom)
Window drain:
  rc = 1/max(denom, eps); ssc = psumW[:, :256]*rc (bf16)
  outp[16, 256] = S16.T @ ssc   (S16[(g,k),g] = 1/8)  -> DMA to HBM
"""

import os
import numpy as np
import ml_dtypes

BF16 = ml_dtypes.bfloat16
FP8 = ml_dtypes.float8_e4m3
FP8_SCALE = 16.0          # h,W1 pre-scaled by 16 before fp8 cast

N_CORES = 8
WSIZE = 16          # graphs per window (16*8 heads = 128 partitions)
H = 8               # heads
F_IN = 256          # in_features
D = 128             # dense dim
HB_W = 257          # 256 feat + 1 (gidx+1) col (doubles as denom source)
MACRO = 8           # slots per macro

_PROGRAM_CACHE = {}


# ----------------------------------------------------------------- host prep
def _preprocess(h, segment_ids, num_graphs):
    N = h.shape[0]
    G = int(num_graphs)
    counts = np.bincount(segment_ids, minlength=G).astype(np.int64)
    g_core = -(-G // N_CORES)
    n_win = -(-g_core // WSIZE)
    starts = np.zeros(G + 1, dtype=np.int64)
    np.cumsum(counts, out=starts[1:])

    # LPT-balance graphs into N_CORES*n_win bins of exactly WSIZE graphs
    n_bins = N_CORES * n_win
    import heapq
    heap = [(0, b, 0) for b in range(n_bins)]   # (load, bin, count)
    heapq.heapify(heap)
    bins = [[] for _ in range(n_bins)]
    for g in np.argsort(-counts, kind="stable"):
        while True:
            load, b, cnt = heapq.heappop(heap)
            if cnt < WSIZE:
                break
        bins[b].append(int(g))
        heapq.heappush(heap, (load + int(counts[g]), b, cnt + 1))
    bin_nodes = np.array([sum(counts[g] for g in bb) for bb in bins])
    t_hi = int(max(1, -(-bin_nodes.max() // 128)))

    # Swap-repair toward heterogeneous windows: per core, shrink n_lo
    # bins to <= (t_hi-1)*128 nodes by swapping graphs with the others
    # (which may grow to <= t_hi*128).  Cuts padding ~2%.
    t_lo = t_hi - 1
    n_lo = 0
    if t_lo >= 1 and n_win > 1:
        cap_lo, cap_hi = t_lo * 128, t_hi * 128
        per_core = [sum(bin_nodes[c * n_win:(c + 1) * n_win])
                    for c in range(N_CORES)]
        max_lo = min((n_win * cap_hi - pc) // (cap_hi - cap_lo)
                     for pc in per_core)
        n_lo = max(0, min(int(max_lo), n_win - 1))
    if n_lo > 0:
        order = np.zeros(n_bins, dtype=np.int64)
        ok_all = True
        for c in range(N_CORES):
            idx = sorted(range(c * n_win, (c + 1) * n_win),
                         key=lambda b: bin_nodes[b])
            lo, hi = idx[:n_lo], idx[n_lo:]
            for L in lo:
                guard = 0
                while bin_nodes[L] > cap_lo and guard < 64:
                    guard += 1
                    best = None        # smallest d covering need
                    part = None        # else largest partial d
                    need = bin_nodes[L] - cap_lo
                    for Hb in hi:
                        room = cap_hi - bin_nodes[Hb]
                        if room <= 0:
                            continue
                        for xi, x in enumerate(bins[L]):
                            for yi, y in enumerate(bins[Hb]):
                                d = counts[x] - counts[y]
                                if d <= 0 or d > room:
                                    continue
                                if d >= need:
                                    if best is None or d < best[0]:
                                        best = (d, L, Hb, xi, yi)
                                elif part is None or d > part[0]:
                                    part = (d, L, Hb, xi, yi)
                    if best is None:
                        best = part
                    if best is None:
                        break
                    d, Lb, Hb, xi, yi = best
                    bins[Lb][xi], bins[Hb][yi] = (bins[Hb][yi],
                                                  bins[Lb][xi])
                    bin_nodes[Lb] -= d
                    bin_nodes[Hb] += d
                if bin_nodes[L] > cap_lo:
                    ok_all = False
            # window order: hi (t_hi) windows first, then lo (t_lo)
            order[c * n_win:(c + 1) * n_win] = idx[n_lo:] + idx[:n_lo]
        if ok_all:
            bins = [bins[b] for b in order]
            tws = [t_hi] * (n_win - n_lo) + [t_lo] * n_lo
        else:
            tws = [t_hi] * n_win
    else:
        tws = [t_hi] * n_win
    woff = np.zeros(n_win + 1, dtype=np.int64)
    np.cumsum([t * 128 for t in tws], out=woff[1:])
    npad = int(woff[-1])

    h32 = np.ascontiguousarray(h, dtype=np.float32)
    hb_all, ht8_all = [], []
    row2graph = np.full((N_CORES, n_win * WSIZE), -1, dtype=np.int64)
    for c in range(N_CORES):
        hb = np.zeros((npad, HB_W), dtype=BF16)
        hb[:, F_IN] = 255.0          # pad rows match no window graph
        hpad = np.zeros((npad, F_IN), dtype=np.float32)
        for w in range(n_win):
            bb = bins[c * n_win + w]
            r = int(woff[w])
            for idx, g in enumerate(bb):
                row2graph[c, w * WSIZE + idx] = g
                n0, n1 = starts[g], starts[g + 1]
                nw = n1 - n0
                if nw == 0:
                    continue
                hpad[r:r + nw] = h32[n0:n1]
                hb[r:r + nw, :F_IN] = h32[n0:n1].astype(BF16)
                hb[r:r + nw, F_IN] = float(idx + 1)   # gidx+1 (denom src)
                r += nw
        hTf = (hpad.T * FP8_SCALE).astype(FP8)     # [F, npad] (w,p,t) order
        hT = np.empty((F_IN, npad), dtype=FP8)     # per-window (t,p) order
        for w in range(n_win):
            blk = hTf[:, woff[w]:woff[w + 1]].reshape(F_IN, 128, tws[w])
            hT[:, woff[w]:woff[w + 1]] = blk.transpose(0, 2, 1).reshape(
                F_IN, -1)
        # DoubleRow packing: ht8[p, i, c] = hT[i*128 + p, c]
        ht8 = np.ascontiguousarray(
            hT.reshape(2, D, npad).transpose(1, 0, 2))
        hb_all.append(hb)
        ht8_all.append(ht8)
    meta = dict(G=G, g_core=g_core, n_win=n_win, tws=tuple(tws),
                npad=npad, row2graph=row2graph)
    return hb_all, ht8_all, meta


def _const_inputs(W1, b1, W2, b2):
    W1 = np.asarray(W1, dtype=np.float32)
    W2 = np.asarray(W2, dtype=np.float32)
    # psw col 256 accumulates (g+1)*d_gk (gidx+1 doubles as denom source);
    # ssc = psw/( (g+1)d ), so fold the (g+1) back in via s16.
    s16 = np.zeros((WSIZE * H, WSIZE), dtype=BF16)
    for g in range(WSIZE):
        s16[g * H:(g + 1) * H, g] = 0.125 * (g + 1)
    w18 = (W1 * FP8_SCALE).astype(FP8)                # [256,128]
    # DoubleRow packing: w1dr[p, i, m] = w18[i*128 + p, m]
    w1dr = np.ascontiguousarray(
        w18.reshape(2, D, D).transpose(1, 0, 2))
    return {
        "w1dr": w1dr,                                            # [128,2,128]
        "w2": np.ascontiguousarray(W2.astype(BF16)),             # [128,8]
        "b1": np.asarray(b1, dtype=np.float32).reshape(D, 1),
        "s16": s16,                                              # [128,16]
        "iota16": np.tile(np.arange(1, WSIZE + 1, dtype=BF16),
                          (WSIZE * H, 1)),
    }


# ------------------------------------------------------------- device program
def _build_program(n_win, tws, npad, num_devices, reps=1, unroll=1):
    import concourse.bacc as bacc
    import concourse.mybir as mybir
    from concourse import tile

    dt = mybir.dt
    AF = mybir.ActivationFunctionType
    tws = list(tws)
    t_max = max(tws)
    woff = np.zeros(n_win + 1, dtype=np.int64)
    np.cumsum([t * 128 for t in tws], out=woff[1:])

    nc = bacc.Bacc("TRN2", target_bir_lowering=False, debug=False,
                   enable_asserts=False, num_devices=num_devices)

    hb_d = nc.dram_tensor("hb", [npad, HB_W], dt.bfloat16,
                          kind="ExternalInput")
    ht8_d = nc.dram_tensor("ht8", [D, 2, npad], dt.float8e4,
                           kind="ExternalInput")
    w1dr_d = nc.dram_tensor("w1dr", [D, 2, D], dt.float8e4,
                            kind="ExternalInput")
    w2_d = nc.dram_tensor("w2", [D, H], dt.bfloat16, kind="ExternalInput")
    b1_d = nc.dram_tensor("b1", [D, 1], dt.float32, kind="ExternalInput")
    s16_d = nc.dram_tensor("s16", [WSIZE * H, WSIZE], dt.bfloat16,
                           kind="ExternalInput")
    iota16_d = nc.dram_tensor("iota16", [WSIZE * H, WSIZE], dt.bfloat16,
                              kind="ExternalInput")
    out_d = nc.dram_tensor("out", [n_win * WSIZE, F_IN], dt.bfloat16,
                           kind="ExternalOutput")

    # per-window blocked view [p, (t f)] and macro slot ranges
    def wview(w):
        return hb_d.ap()[int(woff[w]):int(woff[w + 1]), :].rearrange(
            "(p t) f -> p (t f)", p=128, t=tws[w])

    def wmacros(t_w):
        macros = []
        j0 = 0
        while j0 < t_w:
            macros.append((j0, min(MACRO, t_w - j0)))
            j0 += macros[-1][1]
        return macros

    import contextlib
    with tile.TileContext(nc) as tc:
        with (
            tc.tile_pool(name="consts", bufs=1) as cpool,
            tc.tile_pool(name="hbp", bufs=3) as hbp,
            tc.tile_pool(name="htp", bufs=3) as htp,
            tc.tile_pool(name="actp", bufs=3) as actp,
            tc.tile_pool(name="ep", bufs=3) as epool,
            tc.tile_pool(name="drainp", bufs=2) as drainp,
            tc.tile_pool(name="ps_mm", bufs=2, space="PSUM") as ps_mm,
            tc.tile_pool(name="ps_sco", bufs=2, space="PSUM") as ps_sco,
            tc.tile_pool(name="ps_w", bufs=2, space="PSUM") as ps_w,
            tc.tile_pool(name="ps_out", bufs=2, space="PSUM") as ps_out,
        ):
            w1dr = cpool.tile([D, 2, D], dt.float8e4)
            w2 = cpool.tile([D, H], dt.bfloat16)
            b1 = cpool.tile([D, 1], dt.float32)
            s16 = cpool.tile([WSIZE * H, WSIZE], dt.bfloat16)
            iota16 = cpool.tile([WSIZE * H, WSIZE], dt.bfloat16)
            nc.sync.dma_start(out=iota16[:], in_=iota16_d.ap())
            nc.sync.dma_start(out=w1dr[:], in_=w1dr_d.ap())
            nc.sync.dma_start(out=w2[:], in_=w2_d.ap())
            nc.sync.dma_start(out=b1[:], in_=b1_d.ap())
            nc.sync.dma_start(out=s16[:], in_=s16_d.ap())

            loop_cm = (tc.For_i(0, reps // unroll, 1)
                       if reps // unroll > 1 else contextlib.nullcontext())
            with loop_cm:
              pend_drain = None

              def emit_drain(dr):
                """Window drain, deferred one window for PE overlap."""
                psw_p, w_p = dr
                dcl = drainp.tile([WSIZE * H, 1], dt.float32, tag="dcl")
                nc.vector.tensor_scalar_max(dcl[:],
                                            psw_p[:, F_IN:F_IN + 1], 1e-30)
                rc = drainp.tile([WSIZE * H, 1], dt.float32, tag="rc")
                nc.vector.reciprocal(rc[:], dcl[:])
                ssc = drainp.tile([WSIZE * H, F_IN], dt.bfloat16,
                                  tag="ssc")
                nc.vector.tensor_scalar_mul(ssc[:], psw_p[:, :F_IN], rc[:])
                outp = ps_out.tile([WSIZE, F_IN], dt.float32)
                nc.tensor.matmul(outp[:], s16[:], ssc[:], start=True,
                                 stop=True)
                out_sb = drainp.tile([WSIZE, F_IN], dt.bfloat16, tag="osb")
                nc.vector.tensor_copy(out_sb[:], outp[:])
                nc.scalar.dma_start(
                    out=out_d.ap()[w_p * WSIZE:(w_p + 1) * WSIZE, :],
                    in_=out_sb[:])

              for w in [wi for _ in range(unroll) for wi in range(n_win)]:
                t_w = tws[w]
                B = t_w * 128
                w0 = int(woff[w])
                hb_wv = wview(w)
                hb_sb = hbp.tile([128, t_max, HB_W], dt.bfloat16, tag="hb")
                ht8_sb = htp.tile([D, 2, t_max * 128], dt.float8e4,
                                  tag="h8")
                th = t_w // 2
                nc.gpsimd.dma_start(out=hb_sb[:, :th, :],
                                    in_=hb_wv[:, :th * HB_W])
                nc.gpsimd.dma_start(out=hb_sb[:, th:t_w, :],
                                    in_=hb_wv[:, th * HB_W:])
                bh = B // 2
                nc.sync.dma_start(
                    out=ht8_sb[:, :, :bh],
                    in_=ht8_d.ap()[:, :, w0:w0 + bh])
                nc.sync.dma_start(
                    out=ht8_sb[:, :, bh:B],
                    in_=ht8_d.ap()[:, :, w0 + bh:w0 + B])

                psw = ps_w.tile([WSIZE * H, F_IN + 1], dt.float32)
                pend_sums = None
                for (j0, ns) in wmacros(t_w):
                    # score path in halves of <=512 nodes
                    sco = ps_sco.tile([128, MACRO * H], dt.float32,
                                      tag="sco")
                    n_half = (ns * 128 + 511) // 512
                    for hh in range(n_half):
                        f0 = j0 * 128 + hh * 512
                        fw = min(512, (j0 + ns) * 128 - f0)
                        t1 = ps_mm.tile([D, 512], dt.float32, tag="t1")
                        nc.tensor.matmul(t1[:, :fw], w1dr[:],
                                         ht8_sb[:, :, f0:f0 + fw],
                                         start=True, stop=True,
                                         perf_mode=mybir.MatmulPerfMode
                                         .DoubleRow)
                        a1 = actp.tile([D, 512], dt.bfloat16, tag="a1")
                        nc.scalar.activation(a1[:, :fw], t1[:, :fw],
                                             AF.Tanh, bias=b1[:],
                                             scale=1.0 / (FP8_SCALE ** 2))
                        for jj in range(fw // 128):
                            j = hh * 4 + jj     # slot within macro
                            nc.tensor.matmul(
                                sco[:, j * H:(j + 1) * H],
                                a1[:, jj * 128:(jj + 1) * 128], w2[:],
                                start=True, stop=True)
                    e_sb = epool.tile([128, MACRO * H], dt.bfloat16,
                                      tag="e")
                    nc.scalar.activation(e_sb[:, :ns * H], sco[:, :ns * H],
                                         AF.Exp)
                    msk = epool.tile([128, MACRO * WSIZE], dt.bfloat16,
                                     tag="M")
                    g_b = hb_sb[:, j0:j0 + ns,
                                F_IN:F_IN + 1].broadcast_to(
                                    (128, ns, WSIZE))
                    i_b = iota16[:].unsqueeze(1).broadcast_to(
                        (128, ns, WSIZE))
                    nc.vector.tensor_tensor(
                        msk[:, :ns * WSIZE].rearrange(
                            "p (j g) -> p j g", g=WSIZE),
                        g_b, i_b, mybir.AluOpType.is_equal)
                    em = epool.tile([128, MACRO * 128], dt.bfloat16,
                                    tag="E")
                    e_b = e_sb[:, :ns * H].rearrange(
                        "p (j k) -> p j k", k=H).unsqueeze(2).broadcast_to(
                            (128, ns, WSIZE, H))
                    m_b = msk[:, :ns * WSIZE].rearrange(
                        "p (j g) -> p j g", g=WSIZE).unsqueeze(3).broadcast_to(
                            (128, ns, WSIZE, H))
                    nc.vector.tensor_mul(
                        em[:, :ns * 128].rearrange(
                            "p (j g k) -> p j g k", g=WSIZE, k=H),
                        m_b, e_b)
                    if pend_drain is not None:
                        emit_drain(pend_drain)
                        pend_drain = None
                    if os.environ.get("DEFER_SUMS", "1") == "1":
                        if pend_sums is not None:
                            for jj in range(pend_sums[1]):
                                j = pend_sums[2] + jj
                                nc.tensor.matmul(
                                    psw[:],
                                    pend_sums[0][:,
                                                 jj * 128:(jj + 1) * 128],
                                    hb_sb[:, j, :F_IN + 1],
                                    start=(j == 0), stop=False)
                        pend_sums = (em, ns, j0)
                    else:
                        for jj in range(ns):
                            j = j0 + jj
                            nc.tensor.matmul(
                                psw[:], em[:, jj * 128:(jj + 1) * 128],
                                hb_sb[:, j, :F_IN + 1],
                                start=(j == 0), stop=(j == t_w - 1))

                if pend_sums is not None:
                    # final macro's sums (deferred)
                    for jj in range(pend_sums[1]):
                        j = pend_sums[2] + jj
                        nc.tensor.matmul(
                            psw[:],
                            pend_sums[0][:, jj * 128:(jj + 1) * 128],
                            hb_sb[:, j, :F_IN + 1],
                            start=(j == 0), stop=(j == t_w - 1))
                if os.environ.get("DEFER_DRAIN", "1") == "1":
                    pend_drain = (psw, w)
                else:
                    emit_drain((psw, w))
              if pend_drain is not None:
                emit_drain(pend_drain)
                pend_drain = None

    nc.compile()
    return nc


# ---------------------------------------------------------------- jit runner
class _Runner:
    """Persistent sharded jit wrapper around the compiled Bass program.

    Mirrors bass2jax.run_bass_via_pjrt's multi-core path, but keeps the
    jitted callable and device-resident inputs so repeated executions (for
    timing) skip retrace/recompile/re-transfer.
    """

    def __init__(self, nc):
        import jax
        import concourse.mybir as mybir
        from concourse import bass2jax
        from jax.experimental.shard_map import shard_map
        from jax.sharding import Mesh, PartitionSpec

        bass2jax.install_neuronx_cc_hook()
        self.jax = jax
        part_name = (nc.partition_id_tensor.name
                     if nc.partition_id_tensor else None)
        in_names, out_names, out_avals, zero_outs = [], [], [], []
        for alloc in nc.m.functions[0].allocations:
            if not isinstance(alloc, mybir.MemoryLocationSet):
                continue
            name = alloc.memorylocations[0].name
            if alloc.kind == "ExternalInput":
                if name == part_name:
                    continue
                in_names.append(name)
            elif alloc.kind == "ExternalOutput":
                out_names.append(name)
                shape = tuple(alloc.tensor_shape)
                dtype = mybir.dt.np(alloc.dtype)
                out_avals.append(jax.core.ShapedArray(shape, dtype))
                zero_outs.append(np.zeros(shape, dtype))
        n_params = len(in_names)
        self.in_names = list(in_names)
        self.out_names = out_names
        self.out_avals = out_avals
        self.zero_outs = zero_outs

        bind_names = list(in_names) + list(out_names)
        if part_name is not None:
            bind_names.append(part_name)

        def _body(*args):
            operands = list(args)
            if part_name is not None:
                operands.append(bass2jax.partition_id_tensor())
            outs = bass2jax._bass_exec_p.bind(
                *operands,
                out_avals=tuple(out_avals),
                in_names=tuple(bind_names),
                out_names=tuple(out_names),
                lowering_input_output_aliases=(),
                sim_require_finite=True,
                sim_require_nnan=True,
                nc=nc,
            )
            return tuple(outs)

        devices = jax.devices()[:N_CORES]
        self.mesh = Mesh(np.asarray(devices), ("core",))
        self.pspec = PartitionSpec("core")
        in_specs = (self.pspec,) * (n_params + len(out_names))
        out_specs = (self.pspec,) * len(out_names)
        donate = tuple(range(n_params, n_params + len(out_names)))
        self.sharded = jax.jit(
            shard_map(_body, mesh=self.mesh, in_specs=in_specs,
                      out_specs=out_specs, check_rep=False),
            donate_argnums=donate, keep_unused=True)

    def put_inputs(self, in_maps):
        import jax
        from jax.sharding import NamedSharding
        sh = NamedSharding(self.mesh, self.pspec)
        self.dev_in = [
            jax.device_put(
                np.concatenate([np.asarray(m[name]) for m in in_maps],
                               axis=0), sh)
            for name in self.in_names]

    def run(self, block=True):
        import jax
        from jax.sharding import NamedSharding
        sh = NamedSharding(self.mesh, self.pspec)
        zeros = [jax.device_put(
            np.zeros((N_CORES * z.shape[0], *z.shape[1:]), z.dtype), sh)
            for z in self.zero_outs]
        out = self.sharded(*self.dev_in, *zeros)
        if block:
            jax.block_until_ready(out)
        return out

    def timed_burst(self, n):
        """Dispatch n executions async, block at the end; return wall s."""
        import jax
        import time as _t
        t0 = _t.perf_counter()
        out = None
        for _ in range(n):
            out = self.run(block=False)
        jax.block_until_ready(out)
        return _t.perf_counter() - t0

    def results(self, out_arrs):
        return [
            {name: np.asarray(out_arrs[i]).reshape(
                N_CORES, *self.out_avals[i].shape)[c]
             for i, name in enumerate(self.out_names)}
            for c in range(N_CORES)]


_RUNNER_CACHE = {}


# ------------------------------------------------------------------- kernel()
def kernel(h, segment_ids, W1, b1, W2, b2, num_graphs):
    h = np.asarray(h)
    segment_ids = np.asarray(segment_ids)
    G = int(num_graphs)

    hb_all, ht8_all, meta = _preprocess(h, segment_ids, G)
    consts = _const_inputs(W1, b1, W2, b2)

    key = (meta["n_win"], meta["tws"], meta["npad"])
    if key not in _RUNNER_CACHE:
        nc = _build_program(meta["n_win"], meta["tws"], meta["npad"],
                            N_CORES)
        _RUNNER_CACHE[key] = _Runner(nc)
    runner = _RUNNER_CACHE[key]

    in_maps = []
    for c in range(N_CORES):
        m = {"hb": hb_all[c], "ht8": ht8_all[c]}
        m.update(consts)
        in_maps.append(m)
    runner.put_inputs(in_maps)

    out_arrs = runner.run()   # first call compiles NEFF
    reps = int(os.environ.get("KERNEL_TIME_REPS", "0"))
    if reps:
        n_lo, n_hi = 2, 2 + reps
        t_lo = min(runner.timed_burst(n_lo) for _ in range(3))
        t_hi = min(runner.timed_burst(n_hi) for _ in range(3))
        slope = (t_hi - t_lo) / (n_hi - n_lo)
        print(f"burst timing: n={n_lo}: {t_lo*1e3:.2f} ms, "
              f"n={n_hi}: {t_hi*1e3:.2f} ms")
        print(f"HW exec time: {int(slope * 1e9)} ns")

    res = runner.results(out_arrs)
    out = np.zeros((G, F_IN), dtype=np.float32)
    r2g = meta["row2graph"]
    for c in range(N_CORES):
        valid = r2g[c] >= 0
        out[r2g[c][valid]] = res[c]["out"][valid]
    return out



# revision 50
# speedup vs baseline: 1.6620x; 1.0107x over previous
"""AttnPooling Trainium2 Bass kernel (8-core SPMD).

Math (per graph g, head k):
  scores = tanh(h@W1+b1)@W2+b2                  [N, 8]
  e      = exp(scores)            (no max-sub; scores are O(5), safe)
  s_gk   = sum_{i in g} e_ik * h_i              [G, 8, 256]
  d_gk   = sum_{i in g} e_ik                    [G, 8]
  out_g  = (1/8) sum_k s_gk / d_gk              [G, 256]

Sharding: graphs are LPT-packed into windows of 16 (16 graphs x 8 heads
= 128 PSUM partitions); each of the 8 cores takes n_win windows; window
node counts are padded to a multiple of 128 so every tile belongs to
exactly one window and the program structure is identical across cores
(only the data differs -> single SPMD program).

h ships in two layouts: node-major bf16 (hb, for the weighted-sum
matmul; bf16 needed for the 2e-2 output tolerance) and feature-major
fp8e4m3 x16-scaled DoubleRow-packed (ht8, score path only, where fp8
noise only perturbs attention weights by ~1%).  This is DMA-bound at
~39.5 MB/core; fp8 halves the score-path bytes vs the bf16 baseline.

Per 128-node tile:
  fc1:  t1[128d, F] = W1dr.T @ ht8   (ONE fp8 DoubleRow matmul,
        256-deep contraction at 0.5 cyc/row)
  tanh: a1 = tanh(t1/256 + b1)               (ACT, bf16 out)
  fc2:  sco[128n, 8] = a1_slice.T @ W2       (a1 is the stationary)
  exp:  e = exp(sco + b2)                    (ACT, node-major bf16)
  E[i, g*8+k] = M[i,g] * e[i,k]              (DVE broadcast multiply)
  psumW[128, 257] += E.T @ hb[:, 0:257]      (col 256 of hb is gidx+1, so
                                              psumW[:,256] = (g+1)*denom)
Window drain:
  rc = 1/max((g+1)*denom, eps); ssc = psumW[:, :256]*rc (bf16)
  outp[16, 256] = S16.T @ ssc  (S16[(g,k),g] = (g+1)/8 folds the (g+1)
                                back out)  -> DMA to HBM
Windows are heterogeneous (mix of 25- and 24-tile windows, core-balanced
two-level LPT + swap repair) to cut node padding to ~0.1%.
"""

import os
import numpy as np
import ml_dtypes

BF16 = ml_dtypes.bfloat16
FP8 = ml_dtypes.float8_e4m3
FP8_SCALE = 16.0          # h,W1 pre-scaled by 16 before fp8 cast

N_CORES = 8
WSIZE = 16          # graphs per window (16*8 heads = 128 partitions)
H = 8               # heads
F_IN = 256          # in_features
D = 128             # dense dim
HB_W = 257          # 256 feat + 1 (gidx+1) col (doubles as denom source)
MACRO = 8           # slots per macro

_PROGRAM_CACHE = {}


# ----------------------------------------------------------------- host prep
def _preprocess(h, segment_ids, num_graphs):
    N = h.shape[0]
    G = int(num_graphs)
    counts = np.bincount(segment_ids, minlength=G).astype(np.int64)
    g_core = -(-G // N_CORES)
    n_win = -(-g_core // WSIZE)
    starts = np.zeros(G + 1, dtype=np.int64)
    np.cumsum(counts, out=starts[1:])

    # Two-level LPT: first balance nodes across cores (tightens the
    # per-core max, enabling more small windows below), then LPT within
    # each core into n_win bins of exactly WSIZE graphs.
    n_bins = N_CORES * n_win
    import heapq
    g_per_core = n_win * WSIZE
    cheap = [(0, c, 0) for c in range(N_CORES)]
    heapq.heapify(cheap)
    core_gs = [[] for _ in range(N_CORES)]
    order_desc = np.argsort(-counts, kind="stable")
    for g in order_desc:
        while True:
            load, c, cnt = heapq.heappop(cheap)
            if cnt < g_per_core:
                break
        core_gs[c].append(int(g))
        heapq.heappush(cheap, (load + int(counts[g]), c, cnt + 1))
    bins = [None] * n_bins
    for c in range(N_CORES):
        heap = [(0, b, 0) for b in range(n_win)]
        heapq.heapify(heap)
        cb = [[] for _ in range(n_win)]
        for g in core_gs[c]:            # already size-descending
            while True:
                load, b, cnt = heapq.heappop(heap)
                if cnt < WSIZE:
                    break
            cb[b].append(int(g))
            heapq.heappush(heap, (load + int(counts[g]), b, cnt + 1))
        for b in range(n_win):
            bins[c * n_win + b] = cb[b]
    bin_nodes = np.array([sum(counts[g] for g in bb) for bb in bins])
    t_hi = int(max(1, -(-bin_nodes.max() // 128)))

    # Swap-repair toward heterogeneous windows: per core, shrink n_lo
    # bins to <= (t_hi-1)*128 nodes by swapping graphs with the others
    # (which may grow to <= t_hi*128).  Cuts padding ~2%.
    t_lo = t_hi - 1
    n_lo = 0
    if t_lo >= 1 and n_win > 1:
        cap_lo, cap_hi = t_lo * 128, t_hi * 128
        per_core = [sum(bin_nodes[c * n_win:(c + 1) * n_win])
                    for c in range(N_CORES)]
        max_lo = min((n_win * cap_hi - pc) // (cap_hi - cap_lo)
                     for pc in per_core)
        n_lo = max(0, min(int(max_lo), n_win - 1))
    if n_lo > 0:
        order = np.zeros(n_bins, dtype=np.int64)
        ok_all = True
        for c in range(N_CORES):
            idx = sorted(range(c * n_win, (c + 1) * n_win),
                         key=lambda b: bin_nodes[b])
            lo, hi = idx[:n_lo], idx[n_lo:]
            for L in lo:
                guard = 0
                while bin_nodes[L] > cap_lo and guard < 64:
                    guard += 1
                    best = None        # smallest d covering need
                    part = None        # else largest partial d
                    need = bin_nodes[L] - cap_lo
                    for Hb in hi:
                        room = cap_hi - bin_nodes[Hb]
                        if room <= 0:
                            continue
                        for xi, x in enumerate(bins[L]):
                            for yi, y in enumerate(bins[Hb]):
                                d = counts[x] - counts[y]
                                if d <= 0 or d > room:
                                    continue
                                if d >= need:
                                    if best is None or d < best[0]:
                                        best = (d, L, Hb, xi, yi)
                                elif part is None or d > part[0]:
                                    part = (d, L, Hb, xi, yi)
                    if best is None:
                        best = part
                    if best is None:
                        break
                    d, Lb, Hb, xi, yi = best
                    bins[Lb][xi], bins[Hb][yi] = (bins[Hb][yi],
                                                  bins[Lb][xi])
                    bin_nodes[Lb] -= d
                    bin_nodes[Hb] += d
                if bin_nodes[L] > cap_lo:
                    ok_all = False
            # window order: hi (t_hi) windows first, then lo (t_lo)
            order[c * n_win:(c + 1) * n_win] = idx[n_lo:] + idx[:n_lo]
        if ok_all:
            bins = [bins[b] for b in order]
            tws = [t_hi] * (n_win - n_lo) + [t_lo] * n_lo
        else:
            tws = [t_hi] * n_win
    else:
        tws = [t_hi] * n_win
    woff = np.zeros(n_win + 1, dtype=np.int64)
    np.cumsum([t * 128 for t in tws], out=woff[1:])
    npad = int(woff[-1])

    h32 = np.ascontiguousarray(h, dtype=np.float32)
    hb_all, ht8_all = [], []
    row2graph = np.full((N_CORES, n_win * WSIZE), -1, dtype=np.int64)
    for c in range(N_CORES):
        hb = np.zeros((npad, HB_W), dtype=BF16)
        hb[:, F_IN] = 255.0          # pad rows match no window graph
        hpad = np.zeros((npad, F_IN), dtype=np.float32)
        for w in range(n_win):
            bb = bins[c * n_win + w]
            r = int(woff[w])
            for idx, g in enumerate(bb):
                row2graph[c, w * WSIZE + idx] = g
                n0, n1 = starts[g], starts[g + 1]
                nw = n1 - n0
                if nw == 0:
                    continue
                hpad[r:r + nw] = h32[n0:n1]
                hb[r:r + nw, :F_IN] = h32[n0:n1].astype(BF16)
                hb[r:r + nw, F_IN] = float(idx + 1)   # gidx+1 (denom src)
                r += nw
        hTf = (hpad.T * FP8_SCALE).astype(FP8)     # [F, npad] (w,p,t) order
        hT = np.empty((F_IN, npad), dtype=FP8)     # per-window (t,p) order
        for w in range(n_win):
            blk = hTf[:, woff[w]:woff[w + 1]].reshape(F_IN, 128, tws[w])
            hT[:, woff[w]:woff[w + 1]] = blk.transpose(0, 2, 1).reshape(
                F_IN, -1)
        # DoubleRow packing: ht8[p, i, c] = hT[i*128 + p, c]
        ht8 = np.ascontiguousarray(
            hT.reshape(2, D, npad).transpose(1, 0, 2))
        hb_all.append(hb)
        ht8_all.append(ht8)
    meta = dict(G=G, g_core=g_core, n_win=n_win, tws=tuple(tws),
                npad=npad, row2graph=row2graph)
    return hb_all, ht8_all, meta


def _const_inputs(W1, b1, W2, b2):
    W1 = np.asarray(W1, dtype=np.float32)
    W2 = np.asarray(W2, dtype=np.float32)
    # psw col 256 accumulates (g+1)*d_gk (gidx+1 doubles as denom source);
    # ssc = psw/( (g+1)d ), so fold the (g+1) back in via s16.
    s16 = np.zeros((WSIZE * H, WSIZE), dtype=BF16)
    for g in range(WSIZE):
        s16[g * H:(g + 1) * H, g] = 0.125 * (g + 1)
    w18 = (W1 * FP8_SCALE).astype(FP8)                # [256,128]
    # DoubleRow packing: w1dr[p, i, m] = w18[i*128 + p, m]
    w1dr = np.ascontiguousarray(
        w18.reshape(2, D, D).transpose(1, 0, 2))
    return {
        "w1dr": w1dr,                                            # [128,2,128]
        "w2": np.ascontiguousarray(W2.astype(BF16)),             # [128,8]
        "b1": np.asarray(b1, dtype=np.float32).reshape(D, 1),
        "s16": s16,                                              # [128,16]
        "iota16": np.tile(np.arange(1, WSIZE + 1, dtype=BF16),
                          (WSIZE * H, 1)),
    }


# ------------------------------------------------------------- device program
def _build_program(n_win, tws, npad, num_devices, reps=1, unroll=1):
    import concourse.bacc as bacc
    import concourse.mybir as mybir
    from concourse import tile

    dt = mybir.dt
    AF = mybir.ActivationFunctionType
    tws = list(tws)
    t_max = max(tws)
    woff = np.zeros(n_win + 1, dtype=np.int64)
    np.cumsum([t * 128 for t in tws], out=woff[1:])

    nc = bacc.Bacc("TRN2", target_bir_lowering=False, debug=False,
                   enable_asserts=False, num_devices=num_devices)

    hb_d = nc.dram_tensor("hb", [npad, HB_W], dt.bfloat16,
                          kind="ExternalInput")
    ht8_d = nc.dram_tensor("ht8", [D, 2, npad], dt.float8e4,
                           kind="ExternalInput")
    w1dr_d = nc.dram_tensor("w1dr", [D, 2, D], dt.float8e4,
                            kind="ExternalInput")
    w2_d = nc.dram_tensor("w2", [D, H], dt.bfloat16, kind="ExternalInput")
    b1_d = nc.dram_tensor("b1", [D, 1], dt.float32, kind="ExternalInput")
    s16_d = nc.dram_tensor("s16", [WSIZE * H, WSIZE], dt.bfloat16,
                           kind="ExternalInput")
    iota16_d = nc.dram_tensor("iota16", [WSIZE * H, WSIZE], dt.bfloat16,
                              kind="ExternalInput")
    out_d = nc.dram_tensor("out", [n_win * WSIZE, F_IN], dt.bfloat16,
                           kind="ExternalOutput")

    # per-window blocked view [p, (t f)] and macro slot ranges
    def wview(w):
        return hb_d.ap()[int(woff[w]):int(woff[w + 1]), :].rearrange(
            "(p t) f -> p (t f)", p=128, t=tws[w])

    def wmacros(t_w):
        macros = []
        j0 = 0
        while j0 < t_w:
            macros.append((j0, min(MACRO, t_w - j0)))
            j0 += macros[-1][1]
        return macros

    import contextlib
    with tile.TileContext(nc) as tc:
        with (
            tc.tile_pool(name="consts", bufs=1) as cpool,
            tc.tile_pool(name="hbp", bufs=3) as hbp,
            tc.tile_pool(name="htp", bufs=3) as htp,
            tc.tile_pool(name="actp", bufs=3) as actp,
            tc.tile_pool(name="ep", bufs=3) as epool,
            tc.tile_pool(name="drainp", bufs=2) as drainp,
            tc.tile_pool(name="ps_mm", bufs=2, space="PSUM") as ps_mm,
            tc.tile_pool(name="ps_sco", bufs=2, space="PSUM") as ps_sco,
            tc.tile_pool(name="ps_w", bufs=2, space="PSUM") as ps_w,
            tc.tile_pool(name="ps_out", bufs=2, space="PSUM") as ps_out,
        ):
            w1dr = cpool.tile([D, 2, D], dt.float8e4)
            w2 = cpool.tile([D, H], dt.bfloat16)
            b1 = cpool.tile([D, 1], dt.float32)
            s16 = cpool.tile([WSIZE * H, WSIZE], dt.bfloat16)
            iota16 = cpool.tile([WSIZE * H, WSIZE], dt.bfloat16)
            nc.sync.dma_start(out=iota16[:], in_=iota16_d.ap())
            nc.sync.dma_start(out=w1dr[:], in_=w1dr_d.ap())
            nc.sync.dma_start(out=w2[:], in_=w2_d.ap())
            nc.sync.dma_start(out=b1[:], in_=b1_d.ap())
            nc.sync.dma_start(out=s16[:], in_=s16_d.ap())

            loop_cm = (tc.For_i(0, reps // unroll, 1)
                       if reps // unroll > 1 else contextlib.nullcontext())
            with loop_cm:
              pend_drain = None

              def emit_drain(dr):
                """Window drain, deferred one window for PE overlap."""
                psw_p, w_p = dr
                dcl = drainp.tile([WSIZE * H, 1], dt.float32, tag="dcl")
                nc.vector.tensor_scalar_max(dcl[:],
                                            psw_p[:, F_IN:F_IN + 1], 1e-30)
                rc = drainp.tile([WSIZE * H, 1], dt.float32, tag="rc")
                nc.vector.reciprocal(rc[:], dcl[:])
                ssc = drainp.tile([WSIZE * H, F_IN], dt.bfloat16,
                                  tag="ssc")
                nc.vector.tensor_scalar_mul(ssc[:], psw_p[:, :F_IN], rc[:])
                outp = ps_out.tile([WSIZE, F_IN], dt.float32)
                nc.tensor.matmul(outp[:], s16[:], ssc[:], start=True,
                                 stop=True)
                out_sb = drainp.tile([WSIZE, F_IN], dt.bfloat16, tag="osb")
                nc.vector.tensor_copy(out_sb[:], outp[:])
                nc.scalar.dma_start(
                    out=out_d.ap()[w_p * WSIZE:(w_p + 1) * WSIZE, :],
                    in_=out_sb[:])

              for w in [wi for _ in range(unroll) for wi in range(n_win)]:
                t_w = tws[w]
                B = t_w * 128
                w0 = int(woff[w])
                hb_wv = wview(w)
                hb_sb = hbp.tile([128, t_max, HB_W], dt.bfloat16, tag="hb")
                ht8_sb = htp.tile([D, 2, t_max * 128], dt.float8e4,
                                  tag="h8")
                th = t_w // 2
                nc.gpsimd.dma_start(out=hb_sb[:, :th, :],
                                    in_=hb_wv[:, :th * HB_W])
                nc.gpsimd.dma_start(out=hb_sb[:, th:t_w, :],
                                    in_=hb_wv[:, th * HB_W:])
                bh = B // 2
                nc.sync.dma_start(
                    out=ht8_sb[:, :, :bh],
                    in_=ht8_d.ap()[:, :, w0:w0 + bh])
                nc.sync.dma_start(
                    out=ht8_sb[:, :, bh:B],
                    in_=ht8_d.ap()[:, :, w0 + bh:w0 + B])

                psw = ps_w.tile([WSIZE * H, F_IN + 1], dt.float32)
                pend_sums = None
                for (j0, ns) in wmacros(t_w):
                    # score path in halves of <=512 nodes
                    sco = ps_sco.tile([128, MACRO * H], dt.float32,
                                      tag="sco")
                    n_half = (ns * 128 + 511) // 512
                    for hh in range(n_half):
                        f0 = j0 * 128 + hh * 512
                        fw = min(512, (j0 + ns) * 128 - f0)
                        t1 = ps_mm.tile([D, 512], dt.float32, tag="t1")
                        nc.tensor.matmul(t1[:, :fw], w1dr[:],
                                         ht8_sb[:, :, f0:f0 + fw],
                                         start=True, stop=True,
                                         perf_mode=mybir.MatmulPerfMode
                                         .DoubleRow)
                        a1 = actp.tile([D, 512], dt.bfloat16, tag="a1")
                        nc.scalar.activation(a1[:, :fw], t1[:, :fw],
                                             AF.Tanh, bias=b1[:],
                                             scale=1.0 / (FP8_SCALE ** 2))
                        for jj in range(fw // 128):
                            j = hh * 4 + jj     # slot within macro
                            nc.tensor.matmul(
                                sco[:, j * H:(j + 1) * H],
                                a1[:, jj * 128:(jj + 1) * 128], w2[:],
                                start=True, stop=True)
                    e_sb = epool.tile([128, MACRO * H], dt.bfloat16,
                                      tag="e")
                    nc.scalar.activation(e_sb[:, :ns * H], sco[:, :ns * H],
                                         AF.Exp)
                    msk = epool.tile([128, MACRO * WSIZE], dt.bfloat16,
                                     tag="M")
                    g_b = hb_sb[:, j0:j0 + ns,
                                F_IN:F_IN + 1].broadcast_to(
                                    (128, ns, WSIZE))
                    i_b = iota16[:].unsqueeze(1).broadcast_to(
                        (128, ns, WSIZE))
                    nc.vector.tensor_tensor(
                        msk[:, :ns * WSIZE].rearrange(
                            "p (j g) -> p j g", g=WSIZE),
                        g_b, i_b, mybir.AluOpType.is_equal)
                    em = epool.tile([128, MACRO * 128], dt.bfloat16,
                                    tag="E")
                    e_b = e_sb[:, :ns * H].rearrange(
                        "p (j k) -> p j k", k=H).unsqueeze(2).broadcast_to(
                            (128, ns, WSIZE, H))
                    m_b = msk[:, :ns * WSIZE].rearrange(
                        "p (j g) -> p j g", g=WSIZE).unsqueeze(3).broadcast_to(
                            (128, ns, WSIZE, H))
                    nc.vector.tensor_mul(
                        em[:, :ns * 128].rearrange(
                            "p (j g k) -> p j g k", g=WSIZE, k=H),
                        m_b, e_b)
                    if pend_drain is not None:
                        emit_drain(pend_drain)
                        pend_drain = None
                    if os.environ.get("DEFER_SUMS", "1") == "1":
                        if pend_sums is not None:
                            for jj in range(pend_sums[1]):
                                j = pend_sums[2] + jj
                                nc.tensor.matmul(
                                    psw[:],
                                    pend_sums[0][:,
                                                 jj * 128:(jj + 1) * 128],
                                    hb_sb[:, j, :F_IN + 1],
                                    start=(j == 0), stop=False)
                        pend_sums = (em, ns, j0)
                    else:
                        for jj in range(ns):
                            j = j0 + jj
                            nc.tensor.matmul(
                                psw[:], em[:, jj * 128:(jj + 1) * 128],
                                hb_sb[:, j, :F_IN + 1],
                                start=(j == 0), stop=(j == t_w - 1))

                if pend_sums is not None:
                    # final macro's sums (deferred)
                    for jj in range(pend_sums[1]):
                        j = pend_sums[2] + jj
                        nc.tensor.matmul(
                            psw[:],
                            pend_sums[0][:, jj * 128:(jj + 1) * 128],
                            hb_sb[:, j, :F_IN + 1],
                            start=(j == 0), stop=(j == t_w - 1))
                if os.environ.get("DEFER_DRAIN", "1") == "1":
                    pend_drain = (psw, w)
                else:
                    emit_drain((psw, w))
              if pend_drain is not None:
                emit_drain(pend_drain)
                pend_drain = None

    nc.compile()
    return nc


# ---------------------------------------------------------------- jit runner
class _Runner:
    """Persistent sharded jit wrapper around the compiled Bass program.

    Mirrors bass2jax.run_bass_via_pjrt's multi-core path, but keeps the
    jitted callable and device-resident inputs so repeated executions (for
    timing) skip retrace/recompile/re-transfer.
    """

    def __init__(self, nc):
        import jax
        import concourse.mybir as mybir
        from concourse import bass2jax
        from jax.experimental.shard_map import shard_map
        from jax.sharding import Mesh, PartitionSpec

        bass2jax.install_neuronx_cc_hook()
        self.jax = jax
        part_name = (nc.partition_id_tensor.name
                     if nc.partition_id_tensor else None)
        in_names, out_names, out_avals, zero_outs = [], [], [], []
        for alloc in nc.m.functions[0].allocations:
            if not isinstance(alloc, mybir.MemoryLocationSet):
                continue
            name = alloc.memorylocations[0].name
            if alloc.kind == "ExternalInput":
                if name == part_name:
                    continue
                in_names.append(name)
            elif alloc.kind == "ExternalOutput":
                out_names.append(name)
                shape = tuple(alloc.tensor_shape)
                dtype = mybir.dt.np(alloc.dtype)
                out_avals.append(jax.core.ShapedArray(shape, dtype))
                zero_outs.append(np.zeros(shape, dtype))
        n_params = len(in_names)
        self.in_names = list(in_names)
        self.out_names = out_names
        self.out_avals = out_avals
        self.zero_outs = zero_outs

        bind_names = list(in_names) + list(out_names)
        if part_name is not None:
            bind_names.append(part_name)

        def _body(*args):
            operands = list(args)
            if part_name is not None:
                operands.append(bass2jax.partition_id_tensor())
            outs = bass2jax._bass_exec_p.bind(
                *operands,
                out_avals=tuple(out_avals),
                in_names=tuple(bind_names),
                out_names=tuple(out_names),
                lowering_input_output_aliases=(),
                sim_require_finite=True,
                sim_require_nnan=True,
                nc=nc,
            )
            return tuple(outs)

        devices = jax.devices()[:N_CORES]
        self.mesh = Mesh(np.asarray(devices), ("core",))
        self.pspec = PartitionSpec("core")
        in_specs = (self.pspec,) * (n_params + len(out_names))
        out_specs = (self.pspec,) * len(out_names)
        donate = tuple(range(n_params, n_params + len(out_names)))
        self.sharded = jax.jit(
            shard_map(_body, mesh=self.mesh, in_specs=in_specs,
                      out_specs=out_specs, check_rep=False),
            donate_argnums=donate, keep_unused=True)

    def put_inputs(self, in_maps):
        import jax
        from jax.sharding import NamedSharding
        sh = NamedSharding(self.mesh, self.pspec)
        self.dev_in = [
            jax.device_put(
                np.concatenate([np.asarray(m[name]) for m in in_maps],
                               axis=0), sh)
            for name in self.in_names]

    def run(self, block=True):
        import jax
        from jax.sharding import NamedSharding
        sh = NamedSharding(self.mesh, self.pspec)
        zeros = [jax.device_put(
            np.zeros((N_CORES * z.shape[0], *z.shape[1:]), z.dtype), sh)
            for z in self.zero_outs]
        out = self.sharded(*self.dev_in, *zeros)
        if block:
            jax.block_until_ready(out)
        return out

    def timed_burst(self, n):
        """Dispatch n executions async, block at the end; return wall s."""
        import jax
        import time as _t
        t0 = _t.perf_counter()
        out = None
        for _ in range(n):
            out = self.run(block=False)
        jax.block_until_ready(out)
        return _t.perf_counter() - t0

    def results(self, out_arrs):
        return [
            {name: np.asarray(out_arrs[i]).reshape(
                N_CORES, *self.out_avals[i].shape)[c]
             for i, name in enumerate(self.out_names)}
            for c in range(N_CORES)]


_RUNNER_CACHE = {}


# ------------------------------------------------------------------- kernel()
def kernel(h, segment_ids, W1, b1, W2, b2, num_graphs):
    h = np.asarray(h)
    segment_ids = np.asarray(segment_ids)
    G = int(num_graphs)

    hb_all, ht8_all, meta = _preprocess(h, segment_ids, G)
    consts = _const_inputs(W1, b1, W2, b2)

    key = (meta["n_win"], meta["tws"], meta["npad"])
    if key not in _RUNNER_CACHE:
        nc = _build_program(meta["n_win"], meta["tws"], meta["npad"],
                            N_CORES)
        _RUNNER_CACHE[key] = _Runner(nc)
    runner = _RUNNER_CACHE[key]

    in_maps = []
    for c in range(N_CORES):
        m = {"hb": hb_all[c], "ht8": ht8_all[c]}
        m.update(consts)
        in_maps.append(m)
    runner.put_inputs(in_maps)

    out_arrs = runner.run()   # first call compiles NEFF
    reps = int(os.environ.get("KERNEL_TIME_REPS", "0"))
    if reps:
        n_lo, n_hi = 2, 2 + reps
        t_lo = min(runner.timed_burst(n_lo) for _ in range(3))
        t_hi = min(runner.timed_burst(n_hi) for _ in range(3))
        slope = (t_hi - t_lo) / (n_hi - n_lo)
        print(f"burst timing: n={n_lo}: {t_lo*1e3:.2f} ms, "
              f"n={n_hi}: {t_hi*1e3:.2f} ms")
        print(f"HW exec time: {int(slope * 1e9)} ns")

    res = runner.results(out_arrs)
    out = np.zeros((G, F_IN), dtype=np.float32)
    r2g = meta["row2graph"]
    for c in range(N_CORES):
        valid = r2g[c] >= 0
        out[r2g[c][valid]] = res[c]["out"][valid]
    return out

